# revision 1
# baseline (speedup 1.0000x reference)
"""Trainium2 Bass kernel for nn_Lookback: causal running-mean over T.

out[b, t, c] = (1/(t+1)) * sum_{s<=t} x[b, s, c],  x: [8, 4096, 1024] fp32.

Sharding: data-parallel over batch B — core b handles x[b] ([4096, 1024]).

Per-core algorithm (T tiled into 32 blocks of P=128 rows, pipelined as two
16-tile segments so segment 1's load/phase-A overlaps segment 0's phase B):
  Phase A: tile column-sums  totals[j, c] = sum_p x_j[p, c]
           as a PSUM accumulation of matmuls with indicator weights E_j.
  Phase B: out_k = tril128 @ x_k + G_k @ totals
           where G_k[j, p] = [j < k] broadcasts the carry (sum of previous
           tile totals) to all 128 rows.  Both weights are 0/1 matrices.
           totals rows of the not-yet-finished segment are zeros (memset),
           and G_k only weights rows j < k, so segment 0 outputs are exact.
  Scale by d[t] = 1/(t+1) during PSUM->SBUF eviction (per-partition scalar,
  alternating DVE / ACT), then DMA to DRAM.

Matmuls use float32r (fp32 bits, 1 cycle/row at N>=256 vs 4 for fp32).
"""

import sys

import numpy as np

sys.path.insert(0, "/opt/trn_rl_repo")

import concourse.bass as bass
import concourse.mybir as mybir
import concourse.tile as tile
from concourse import bacc
from concourse.bass_utils import run_bass_kernel_spmd

B, T, C = 8, 4096, 1024
P = 128
NT = T // P          # 32 row tiles per core
NSEG = 4
SEG = NT // NSEG     # 16 tiles per segment
CH = 512             # PSUM bank chunk (fp32)
NCH = C // CH
F32 = mybir.dt.float32
F32R = mybir.dt.float32r

_cache = {}


def _consts():
    """Host-precomputed weight matrices (shared by all cores)."""
    # trilT[q, p] = [q <= p]  (lhsT of the lower-triangular ones matrix)
    tril_t = np.tril(np.ones((P, P), np.float32)).T.copy()
    # E_all[:, k*NT:(k+1)*NT] = E_k with E_k[p, m] = [m == k] (global row)
    e_all = np.zeros((P, NT * NT), np.float32)
    for k in range(NT):
        e_all[:, k * NT + k] = 1.0
    # G_all[:, k*P:(k+1)*P] = G_k with G_k[j, p] = [j < k]
    g_all = np.zeros((NT, NT * P), np.float32)
    for k in range(NT):
        g_all[:k, k * P:(k + 1) * P] = 1.0
    # recip[p, k] = 1 / (128*k + p + 1)
    t_idx = np.arange(T, dtype=np.float64).reshape(NT, P).T  # [P, NT]
    recip = (1.0 / (t_idx + 1.0)).astype(np.float32)
    return tril_t, e_all, g_all, recip


def _build():
    nc = bacc.Bacc("TRN2", target_bir_lowering=False, debug=False, num_devices=B)
    x_d = nc.dram_tensor("x", [T, C], F32R, kind="ExternalInput").ap()
    tril_d = nc.dram_tensor("tril_t", [P, P], F32R, kind="ExternalInput").ap()
    e_d = nc.dram_tensor("e_all", [P, NT * NT], F32R, kind="ExternalInput").ap()
    g_d = nc.dram_tensor("g_all", [NT, NT * P], F32R, kind="ExternalInput").ap()
    r_d = nc.dram_tensor("recip", [P, NT], F32, kind="ExternalInput").ap()
    out_d = nc.dram_tensor("out", [T, C], F32, kind="ExternalOutput").ap()

    x_t = x_d.rearrange("(n p) c -> n p c", p=P)      # [NT, P, C]
    out_t = out_d.rearrange("(n p) c -> n p c", p=P)

    with tile.TileContext(nc) as tc:
        with (
            tc.tile_pool(name="const", bufs=1) as cp,
            tc.tile_pool(name="xres", bufs=1) as xp,
            tc.tile_pool(name="tot", bufs=1) as tp,
            tc.tile_pool(name="ev", bufs=4) as ep,
            tc.tile_pool(name="ps", bufs=3, space=bass.MemorySpace.PSUM) as psp,
            tc.tile_pool(name="pt", bufs=1, space=bass.MemorySpace.PSUM) as ptp,
        ):
            tril_s = cp.tile([P, P], F32R)
            e_s = cp.tile([P, NT * NT], F32R)
            g_s = cp.tile([NT, NT * P], F32R)
            r_s = cp.tile([P, NT], F32)
            nc.sync.dma_start(tril_s[:], tril_d)
            nc.sync.dma_start(e_s[:], e_d)
            nc.sync.dma_start(g_s[:], g_d)
            nc.sync.dma_start(r_s[:], r_d)

            xr = xp.tile([P, NT * C], F32R)           # resident input
            tot_list = []

            # PE warm-up burst: ~10us of back-to-back dummy matmuls while
            # the first segment streams in, so the HAM clock gate reaches
            # 8/8 (2.4 GHz) before the real matmul streams start.
            dmy = psp.tile([P, CH], F32, tag="ps")
            for _ in range(40):
                nc.tensor.matmul(dmy[:], tril_s[:], e_s[:, 0:CH],
                                 start=True, stop=True)

            for s in range(NSEG):
                k0, k1 = s * SEG, (s + 1) * SEG
                pt = ptp.tile([NT, C], F32)
                # ---- load + phase A for this segment -----------------
                for k in range(k0, k1):
                    xs = xr[:, k * C:(k + 1) * C]
                    nc.sync.dma_start(xs, x_t[k])
                    for h in range(NCH):
                        sl = slice(h * CH, (h + 1) * CH)
                        nc.tensor.matmul(
                            pt[:, sl],
                            e_s[:, k * NT:(k + 1) * NT],
                            xs[:, sl],
                            start=(k == k0),
                            stop=(k == k1 - 1),
                        )
                # per-segment running totals tile: no WAR against the G
                # matmuls of earlier segments (they read their own tile)
                tot_s = tp.tile([NT, C], F32R, tag=f"tot{s}")
                if s == 0:
                    nc.vector.tensor_copy(tot_s[:], pt[:])
                else:
                    nc.vector.tensor_add(tot_s[:], tot_list[s - 1][:], pt[:])
                tot_list.append(tot_s)

                # ---- phase B + scaled eviction + store ---------------
                for k in range(k0, k1):
                    xs = xr[:, k * C:(k + 1) * C]
                    ps = psp.tile([P, C], F32)
                    # both chunks of the tril matmul first (same weights),
                    # then both chunks of the carry matmul
                    for h in range(NCH):
                        sl = slice(h * CH, (h + 1) * CH)
                        nc.tensor.matmul(
                            ps[:, sl], tril_s[:], xs[:, sl],
                            start=True, stop=(k == 0),
                        )
                    if k > 0:
                        for h in range(NCH):
                            sl = slice(h * CH, (h + 1) * CH)
                            nc.tensor.matmul(
                                ps[:, sl], g_s[:, k * P:(k + 1) * P], tot_s[:, sl],
                                start=False, stop=True,
                            )
                    o = ep.tile([P, C], F32)
                    scale = r_s[:, k:k + 1]
                    if k % 2 == 0:
                        nc.vector.tensor_scalar_mul(o[:], ps[:], scale)
                    else:
                        nc.scalar.activation(
                            o[:], ps[:], mybir.ActivationFunctionType.Copy,
                            scale=scale,
                        )
                    nc.sync.dma_start(out_t[k], o[:])

    nc.compile()
    return nc


def _run(x, trace=False):
    x = np.ascontiguousarray(x, dtype=np.float32)
    assert x.shape == (B, T, C)
    if "nc" not in _cache:
        _cache["nc"] = _build()
        _cache["consts"] = _consts()
    nc = _cache["nc"]
    tril_t, e_all, g_all, recip = _cache["consts"]
    in_maps = [
        {"x": x[b], "tril_t": tril_t, "e_all": e_all, "g_all": g_all, "recip": recip}
        for b in range(B)
    ]
    res = run_bass_kernel_spmd(nc, in_maps, core_ids=list(range(B)), trace=trace)
    out = np.stack([res.results[b]["out"] for b in range(B)])
    return out, res


def kernel(x):
    out, _ = _run(x, trace=False)
    return out



# revision 12
# speedup vs baseline: 1.1077x; 1.1077x over previous
"""Trainium2 Bass kernel for nn_Lookback: causal running-mean over T.

out[b, t, c] = (1/(t+1)) * sum_{s<=t} x[b, s, c],  x: [8, 4096, 1024] fp32.

Sharding: data-parallel over batch B — core b handles x[b] ([4096, 1024]).

Per-core algorithm (T tiled into 32 blocks of P=128 rows):
  For each tile k (partition-REVERSED outputs: ps[p] holds global row
  128k + 127 - p, so the full-tile running total lands on partition 0):
    ps_k = flipT @ x_k             (flipT[q, p] = [q <= 127-p]: reversed
                                    within-tile causal prefix sums)
    ps_k += ones1 @ carry          (k>0: K=1 matmul broadcasts the running
                                    total of tiles <k to all 128 rows)
    carry = ps_k[0]                (partition 0 = running total through
                                    tile k; extracted by DVE — a legal
                                    same-partition copy. This replaces a
                                    whole phase-A totals pass.)
    out_k = ps_k * 1/(t+1)         (ACT eviction, per-partition scale,
                                    cast to bf16)
  The host un-reverses each 128-row block during the gather (numpy view).
  Output is stored as bf16 (tolerance 2e-2 >> bf16's ~2e-3) halving store
  traffic: 16 MiB loads + 8 MiB stores ~= 70us HBM floor per core.

Engine split: SP issues loads (queue 1), GPSIMD issues stores (queue 0 —
separate queue so stores never block loads), DVE extracts carry rows,
ACT evicts, PE does matmuls only.
"""

import sys

import numpy as np

sys.path.insert(0, "/opt/trn_rl_repo")

import concourse.bass as bass
import concourse.mybir as mybir
import concourse.tile as tile
from concourse import bacc
from concourse.bass_utils import run_bass_kernel_spmd

B, T, C = 8, 4096, 1024
P = 128
NT = T // P          # 32 row tiles per core
CH = 512             # PSUM bank chunk (fp32)
NCH = C // CH
LB = 4               # tiles per load DMA  (2 MiB fp32)
SB = 2               # tiles per store DMA (512 KiB bf16)
F32 = mybir.dt.float32
F32R = mybir.dt.float32r
BF16 = mybir.dt.bfloat16

_cache = {}


def _consts():
    """Host-precomputed weight matrices (shared by all cores)."""
    # flipT[q, p] = [q <= 127 - p]: out partition p = global row 128k+127-p
    flip_t = np.triu(np.ones((P, P), np.float32))[:, ::-1].copy()
    ones1 = np.ones((1, P), np.float32)
    # recip[p, k] = 1 / (128*k + 127 - p + 1)
    pidx = np.arange(P, dtype=np.float64)[:, None]      # [P, 1]
    kidx = np.arange(NT, dtype=np.float64)[None, :]     # [1, NT]
    recip = (1.0 / (128.0 * kidx + 128.0 - pidx)).astype(np.float32)
    return flip_t, ones1, recip


def _build():
    nc = bacc.Bacc("TRN2", target_bir_lowering=False, debug=False, num_devices=B)
    x_d = nc.dram_tensor("x", [T, C], F32R, kind="ExternalInput").ap()
    flip_d = nc.dram_tensor("flip_t", [P, P], F32R, kind="ExternalInput").ap()
    ones_d = nc.dram_tensor("ones1", [1, P], F32R, kind="ExternalInput").ap()
    r_d = nc.dram_tensor("recip", [P, NT], F32, kind="ExternalInput").ap()
    out_d = nc.dram_tensor("out", [T, C], BF16, kind="ExternalOutput").ap()

    x_g = x_d.rearrange("(j n p) c -> j p n c", p=P, n=LB)     # [8, P, LB, C]
    out_g = out_d.rearrange("(m n p) c -> m p n c", p=P, n=SB)  # [16, P, SB, C]

    with tile.TileContext(nc) as tc:
        with (
            tc.tile_pool(name="const", bufs=1) as cp,
            tc.tile_pool(name="xres", bufs=1) as xp,
            tc.tile_pool(name="carry", bufs=1) as kp,
            tc.tile_pool(name="ev", bufs=4) as ep,
            tc.tile_pool(name="ps", bufs=3, space=bass.MemorySpace.PSUM) as psp,
            tc.tile_pool(name="wu", bufs=1, space=bass.MemorySpace.PSUM) as wup,
        ):
            flip_s = cp.tile([P, P], F32R)
            ones_s = cp.tile([1, P], F32R)
            r_s = cp.tile([P, NT], F32)
            nc.sync.dma_start(flip_s[:], flip_d)
            nc.sync.dma_start(ones_s[:], ones_d)
            nc.sync.dma_start(r_s[:], r_d)

            xr = xp.tile([P, NT, C], F32R)            # resident input
            carry = kp.tile([1, 2, C], F32R)          # running-total row, 2 slots

            # stream all input loads up-front on the sync queue (no deps)
            for j in range(NT // LB):
                nc.sync.dma_start(xr[:, j * LB:(j + 1) * LB, :], x_g[j])

            # PE warm-up: ~3.5us of back-to-back matmuls while the first
            # load streams in, so the HAM clock gate reaches 8/8 (2.4 GHz)
            # before the real matmul stream starts.
            wu = wup.tile([P, P], F32)
            for _ in range(10):
                nc.tensor.matmul(wu[:], flip_s[:], flip_s[:],
                                 start=True, stop=True)

            o = None
            for k in range(NT):
                xs = xr[:, k, :]
                ps = psp.tile([P, C], F32, tag="ps")
                for h in range(NCH):
                    sl = slice(h * CH, (h + 1) * CH)
                    nc.tensor.matmul(
                        ps[:, sl], flip_s[:], xs[:, sl],
                        start=True, stop=(k == 0),
                    )
                if k > 0:
                    for h in range(NCH):
                        sl = slice(h * CH, (h + 1) * CH)
                        nc.tensor.matmul(
                            ps[:, sl], ones_s[:], carry[:, (k - 1) % 2, sl],
                            start=False, stop=True,
                        )
                # extract running total (partition 0) for the next tile,
                # split into chunks so mm2 chunk h of tile k+1 only waits
                # for chunk h of this extract (pipelines the serial chain)
                if k < NT - 1:
                    for h in range(NCH):
                        sl = slice(h * CH, (h + 1) * CH)
                        nc.vector.tensor_copy(carry[:, k % 2, sl], ps[0:1, sl])
                # scaled eviction to bf16 on ACT
                if k % SB == 0:
                    o = ep.tile([P, SB, C], BF16, tag="o")
                nc.scalar.activation(
                    o[:, k % SB, :], ps[:], mybir.ActivationFunctionType.Copy,
                    scale=r_s[:, k:k + 1],
                )
                if k % SB == SB - 1:
                    nc.gpsimd.dma_start(out_g[k // SB], o[:])

    nc.compile()
    return nc


def _run(x, trace=False):
    x = np.ascontiguousarray(x, dtype=np.float32)
    assert x.shape == (B, T, C)
    if "nc" not in _cache:
        _cache["nc"] = _build()
        _cache["consts"] = _consts()
    nc = _cache["nc"]
    flip_t, ones1, recip = _cache["consts"]
    in_maps = [
        {"x": x[b], "flip_t": flip_t, "ones1": ones1, "recip": recip}
        for b in range(B)
    ]
    res = run_bass_kernel_spmd(nc, in_maps, core_ids=list(range(B)), trace=trace)
    # un-reverse each 128-row block (device wrote them partition-flipped)
    out = np.stack([
        np.asarray(res.results[b]["out"])
        .reshape(NT, P, C)[:, ::-1, :]
        .reshape(T, C)
        .astype(np.float32)
        for b in range(B)
    ])
    return out, res


def kernel(x):
    out, _ = _run(x, trace=False)
    return out


# revision 16
# speedup vs baseline: 1.3437x; 1.2131x over previous
"""Trainium2 Bass kernel for nn_Lookback: causal running-mean over T.

out[b, t, c] = (1/(t+1)) * sum_{s<=t} x[b, s, c],  x: [8, 4096, 1024] fp32.

Sharding: data-parallel over batch B — core b handles x[b] ([4096, 1024]).

Per-core algorithm (T tiled into 32 blocks of P=128 rows):
  For each tile k (partition-REVERSED outputs: ps[p] holds global row
  128k + 127 - p, so the full-tile running total lands on partition 0):
    ps_k = flipT @ x_k             (flipT[q, p] = [q <= 127-p]: reversed
                                    within-tile causal prefix sums)
    ps_k += ones1 @ carry          (k>0: K=1 matmul broadcasts the running
                                    total of tiles <k to all 128 rows)
    carry = ps_k[0]                (partition 0 = running total through
                                    tile k; extracted by DVE — a legal
                                    same-partition copy. This replaces a
                                    whole phase-A totals pass.)
    out_k = ps_k * 1/(t+1)         (ACT eviction, per-partition scale,
                                    cast to bf16)
  The host un-reverses each 128-row block during the gather (numpy view).
  Output is stored as bf16 (tolerance 2e-2 >> bf16's ~2e-3) halving store
  traffic: 16 MiB loads + 8 MiB stores ~= 70us HBM floor per core.

Engine split: SP issues loads (queue 1), GPSIMD issues stores (queue 0 —
separate queue so stores never block loads), DVE extracts carry rows,
ACT evicts, PE does matmuls only.
"""

import sys

import numpy as np

sys.path.insert(0, "/opt/trn_rl_repo")

import concourse.bass as bass
import concourse.mybir as mybir
import concourse.tile as tile
from concourse import bacc
from concourse.bass_utils import run_bass_kernel_spmd

B, T, C = 8, 4096, 1024
P = 128
NT = T // P          # 32 row tiles per core
CH = 512             # PSUM bank chunk (fp32)
NCH = C // CH
LB = 4               # tiles per load DMA  (2 MiB fp32)
SB = 2               # tiles per store DMA (512 KiB bf16)
F32 = mybir.dt.float32
F32R = mybir.dt.float32r
BF16 = mybir.dt.bfloat16

_cache = {}


def _consts():
    """Host-precomputed weight matrices (shared by all cores)."""
    # flipT[q, p] = [q <= 127 - p]: out partition p = global row 128k+127-p
    flip_t = np.triu(np.ones((P, P), np.float32))[:, ::-1].copy()
    ones1 = np.ones((1, P), np.float32)
    # recip[p, k] = 1 / (128*k + 127 - p + 1)
    pidx = np.arange(P, dtype=np.float64)[:, None]      # [P, 1]
    kidx = np.arange(NT, dtype=np.float64)[None, :]     # [1, NT]
    recip = (1.0 / (128.0 * kidx + 128.0 - pidx)).astype(np.float32)
    return flip_t, ones1, recip


def _build():
    nc = bacc.Bacc("TRN2", target_bir_lowering=False, debug=False, num_devices=B)
    x_d = nc.dram_tensor("x", [T, C], F32R, kind="ExternalInput").ap()
    flip_d = nc.dram_tensor("flip_t", [P, P], F32R, kind="ExternalInput").ap()
    ones_d = nc.dram_tensor("ones1", [1, P], F32R, kind="ExternalInput").ap()
    r_d = nc.dram_tensor("recip", [P, NT], F32, kind="ExternalInput").ap()
    out_d = nc.dram_tensor("out", [T, C], BF16, kind="ExternalOutput").ap()

    out_g = out_d.rearrange("(m n p) c -> m p n c", p=P, n=SB)  # [16, P, SB, C]

    with tile.TileContext(nc) as tc:
        with (
            tc.tile_pool(name="const", bufs=1) as cp,
            tc.tile_pool(name="xres", bufs=1) as xp,
            tc.tile_pool(name="carry", bufs=1) as kp,
            tc.tile_pool(name="ev", bufs=4) as ep,
            tc.tile_pool(name="ps", bufs=3, space=bass.MemorySpace.PSUM) as psp,
            tc.tile_pool(name="wu", bufs=1, space=bass.MemorySpace.PSUM) as wup,
        ):
            flip_s = cp.tile([P, P], F32R)
            ones_s = cp.tile([1, P], F32R)
            r_s = cp.tile([P, NT], F32)
            nc.sync.dma_start(flip_s[:], flip_d)
            nc.sync.dma_start(ones_s[:], ones_d)
            nc.sync.dma_start(r_s[:], r_d)

            xr = xp.tile([P, NT, C], F32R)            # resident input
            carry = kp.tile([1, 2, C], F32R)          # running-total row, 2 slots

            # stream all input loads up-front on the sync queue (no deps).
            # First tile alone so mm1(0) starts early; last group split in
            # two so the final tiles' compute overlaps the last transfer.
            x_pn = x_d.rearrange("(n p) c -> p n c", p=P)   # [P, NT, C]
            groups = [(0, 1), (1, 3)]
            groups += [(4 * j, 4) for j in range(1, 7)]
            groups += [(28, 2), (30, 2)]
            for k0, n in groups:
                nc.sync.dma_start(xr[:, k0:k0 + n, :], x_pn[:, k0:k0 + n, :])

            # PE warm-up: >3.4us of back-to-back matmuls (the HAM window)
            # bridging until the first load lands, so the clock gate reaches
            # 8/8 (2.4 GHz) before the real matmul stream starts and stays
            # there.
            wu = wup.tile([P, P], F32)
            for _ in range(28):
                nc.tensor.matmul(wu[:], flip_s[:], flip_s[:],
                                 start=True, stop=True)

            o = None
            for k in range(NT):
                xs = xr[:, k, :]
                ps = psp.tile([P, C], F32, tag="ps")
                for h in range(NCH):
                    sl = slice(h * CH, (h + 1) * CH)
                    nc.tensor.matmul(
                        ps[:, sl], flip_s[:], xs[:, sl],
                        start=True, stop=(k == 0),
                    )
                if k > 0:
                    for h in range(NCH):
                        sl = slice(h * CH, (h + 1) * CH)
                        nc.tensor.matmul(
                            ps[:, sl], ones_s[:], carry[:, (k - 1) % 2, sl],
                            start=False, stop=True,
                        )
                # extract running total (partition 0) for the next tile
                if k < NT - 1:
                    nc.vector.tensor_copy(carry[:, k % 2, :], ps[0:1, :])
                # scaled eviction to bf16 on ACT
                if k % SB == 0:
                    o = ep.tile([P, SB, C], BF16, tag="o")
                nc.scalar.activation(
                    o[:, k % SB, :], ps[:], mybir.ActivationFunctionType.Copy,
                    scale=r_s[:, k:k + 1],
                )
                if k % SB == SB - 1:
                    nc.gpsimd.dma_start(out_g[k // SB], o[:])

    nc.compile()
    return nc


def _run(x, trace=False):
    x = np.ascontiguousarray(x, dtype=np.float32)
    assert x.shape == (B, T, C)
    if "nc" not in _cache:
        _cache["nc"] = _build()
        _cache["consts"] = _consts()
    nc = _cache["nc"]
    flip_t, ones1, recip = _cache["consts"]
    in_maps = [
        {"x": x[b], "flip_t": flip_t, "ones1": ones1, "recip": recip}
        for b in range(B)
    ]
    res = run_bass_kernel_spmd(nc, in_maps, core_ids=list(range(B)), trace=trace)
    # un-reverse each 128-row block (device wrote them partition-flipped)
    out = np.stack([
        np.asarray(res.results[b]["out"])
        .reshape(NT, P, C)[:, ::-1, :]
        .reshape(T, C)
        .astype(np.float32)
        for b in range(B)
    ])
    return out, res


def kernel(x):
    out, _ = _run(x, trace=False)
    return out


# revision 17
# speedup vs baseline: 1.4741x; 1.0970x over previous
"""Trainium2 Bass kernel for nn_Lookback: causal running-mean over T.

out[b, t, c] = (1/(t+1)) * sum_{s<=t} x[b, s, c],  x: [8, 4096, 1024] fp32.

Sharding: data-parallel over batch B — core b handles x[b] ([4096, 1024]).

Per-core algorithm (T tiled into 32 blocks of P=128 rows, processed as 16
pairs; partition-REVERSED outputs: ps[p] holds global row 128k + 127 - p,
so each tile's running total lands on partition 0):
  For pair m (tiles a=2m, b=2m+1):
    ps_a = flipT @ x_a            (+ ones1 @ carry[m-1] for m>0)
    ps_b = flipT @ x_b + ones128 @ x_a   (+ ones1 @ carry[m-1])
    carry[m] = ps_b[0]            (partition 0 = running total through
                                   tile b; one DVE extract per PAIR — the
                                   serial carry chain at half frequency)
    out_{a,b} = ps_{a,b} * 1/(t+1)  (ACT eviction, per-partition scale, bf16)
  All matmuls run in bf16 (1 cyc/row; fp32r measures ~2-3 cyc/row on this
  part, and the PE clock is duty-cycle throttled, so bf16 halves PE time).
  x is cast f32->bf16 on-chip (DVE/ACT) through a staging ring; input
  precision bf16 is ~0.2% — tolerance is 2e-2.
  The host un-reverses each 128-row block during the gather (numpy view).
  Output is stored as bf16, halving store traffic: 16 MiB loads + 8 MiB
  stores ~= 60-70us HBM floor per core at ~400 GB/s.

Engine split: SP issues loads (queue 1), GPSIMD issues stores (queue 0 —
separate queue so stores never block loads), DVE casts + extracts carry,
ACT casts + evicts, PE does matmuls only.
"""

import sys

import numpy as np

sys.path.insert(0, "/opt/trn_rl_repo")

import concourse.bass as bass
import concourse.mybir as mybir
import concourse.tile as tile
from concourse import bacc
from concourse.bass_utils import run_bass_kernel_spmd

B, T, C = 8, 4096, 1024
P = 128
NT = T // P          # 32 row tiles per core
NP = NT // 2         # 16 pairs
CH = 512             # PSUM bank chunk (fp32)
NCH = C // CH
F32 = mybir.dt.float32
BF16 = mybir.dt.bfloat16

_cache = {}


def _consts():
    """Host-precomputed weight matrices (shared by all cores)."""
    # flipT[q, p] = [q <= 127 - p]: out partition p = global row 128k+127-p
    flip_t = np.triu(np.ones((P, P), np.float32))[:, ::-1].copy()
    ones1 = np.ones((1, P), np.float32)
    ones128 = np.ones((P, P), np.float32)
    # recip[p, k] = 1 / (128*k + 127 - p + 1)
    pidx = np.arange(P, dtype=np.float64)[:, None]      # [P, 1]
    kidx = np.arange(NT, dtype=np.float64)[None, :]     # [1, NT]
    recip = (1.0 / (128.0 * kidx + 128.0 - pidx)).astype(np.float32)
    import ml_dtypes
    bf = lambda a: a.astype(ml_dtypes.bfloat16)
    return bf(flip_t), bf(ones1), bf(ones128), recip


def _build():
    nc = bacc.Bacc("TRN2", target_bir_lowering=False, debug=False, num_devices=B)
    x_d = nc.dram_tensor("x", [T, C], F32, kind="ExternalInput").ap()
    flip_d = nc.dram_tensor("flip_t", [P, P], BF16, kind="ExternalInput").ap()
    ones1_d = nc.dram_tensor("ones1", [1, P], BF16, kind="ExternalInput").ap()
    ones128_d = nc.dram_tensor("ones128", [P, P], BF16, kind="ExternalInput").ap()
    r_d = nc.dram_tensor("recip", [P, NT], F32, kind="ExternalInput").ap()
    out_d = nc.dram_tensor("out", [T, C], BF16, kind="ExternalOutput").ap()

    x_pn = x_d.rearrange("(n p) c -> p n c", p=P)                # [P, NT, C]
    out_g = out_d.rearrange("(m n p) c -> m p n c", p=P, n=2)    # [16, P, 2, C]

    with tile.TileContext(nc) as tc:
        with (
            tc.tile_pool(name="const", bufs=1) as cp,
            tc.tile_pool(name="stg", bufs=4) as sp,
            tc.tile_pool(name="xbf", bufs=1) as xp,
            tc.tile_pool(name="carry", bufs=1) as kp,
            tc.tile_pool(name="ev", bufs=4) as ep,
            tc.tile_pool(name="ps", bufs=4, space=bass.MemorySpace.PSUM) as psp,
        ):
            flip_s = cp.tile([P, P], BF16)
            ones1_s = cp.tile([1, P], BF16)
            ones128_s = cp.tile([P, P], BF16)
            r_s = cp.tile([P, NT], F32)
            nc.sync.dma_start(flip_s[:], flip_d)
            nc.sync.dma_start(ones1_s[:], ones1_d)
            nc.sync.dma_start(ones128_s[:], ones128_d)
            nc.sync.dma_start(r_s[:], r_d)

            xr = xp.tile([P, NT, C], BF16)            # bf16 resident input
            carry = kp.tile([1, 2, C], BF16)          # running-total row, 2 slots

            # all loads up-front on the sync queue; stage ring (bufs=4)
            # throttles them to stay <= 4 pairs ahead of the casts
            stage = []
            for g in range(NP):
                st = sp.tile([P, 2, C], F32, tag="stg")
                nc.sync.dma_start(st[:], x_pn[:, 2 * g:2 * g + 2, :])
                stage.append(st)

            def cast_pair(g):
                nc.vector.tensor_copy(xr[:, 2 * g, :], stage[g][:, 0, :])
                nc.scalar.activation(
                    xr[:, 2 * g + 1, :], stage[g][:, 1, :],
                    mybir.ActivationFunctionType.Copy,
                )

            # PE warm-up while the first loads+casts land
            wu = psp.tile([P, C], F32, tag="ps")
            for _ in range(12):
                nc.tensor.matmul(wu[:, 0:P], flip_s[:], flip_s[:],
                                 start=True, stop=True)

            cast_pair(0)
            cast_pair(1)

            for m in range(NP):
                a, b = 2 * m, 2 * m + 1
                if m + 2 < NP:
                    cast_pair(m + 2)
                xa = xr[:, a, :]
                xb = xr[:, b, :]
                ps_a = psp.tile([P, C], F32, tag="ps")
                ps_b = psp.tile([P, C], F32, tag="ps")
                for h in range(NCH):
                    sl = slice(h * CH, (h + 1) * CH)
                    nc.tensor.matmul(ps_a[:, sl], flip_s[:], xa[:, sl],
                                     start=True, stop=(m == 0))
                    nc.tensor.matmul(ps_b[:, sl], flip_s[:], xb[:, sl],
                                     start=True, stop=False)
                    nc.tensor.matmul(ps_b[:, sl], ones128_s[:], xa[:, sl],
                                     start=False, stop=(m == 0))
                if m > 0:
                    # carry-dependent matmuls last: ps_b first (the extract
                    # chain hangs off it), ps_a off the critical path
                    for h in range(NCH):
                        sl = slice(h * CH, (h + 1) * CH)
                        nc.tensor.matmul(ps_b[:, sl], ones1_s[:],
                                         carry[:, (m - 1) % 2, sl],
                                         start=False, stop=True)
                    for h in range(NCH):
                        sl = slice(h * CH, (h + 1) * CH)
                        nc.tensor.matmul(ps_a[:, sl], ones1_s[:],
                                         carry[:, (m - 1) % 2, sl],
                                         start=False, stop=True)
                # extract running total (partition 0) for the next pair
                if m < NP - 1:
                    nc.vector.tensor_copy(carry[:, m % 2, :], ps_b[0:1, :])
                # scaled evictions to bf16 on ACT
                o = ep.tile([P, 2, C], BF16, tag="o")
                nc.scalar.activation(
                    o[:, 0, :], ps_a[:], mybir.ActivationFunctionType.Copy,
                    scale=r_s[:, a:a + 1],
                )
                nc.scalar.activation(
                    o[:, 1, :], ps_b[:], mybir.ActivationFunctionType.Copy,
                    scale=r_s[:, b:b + 1],
                )
                nc.gpsimd.dma_start(out_g[m], o[:])

    nc.compile()
    return nc


def _run(x, trace=False):
    x = np.ascontiguousarray(x, dtype=np.float32)
    assert x.shape == (B, T, C)
    if "nc" not in _cache:
        _cache["nc"] = _build()
        _cache["consts"] = _consts()
    nc = _cache["nc"]
    flip_t, ones1, ones128, recip = _cache["consts"]
    in_maps = [
        {"x": x[b], "flip_t": flip_t, "ones1": ones1, "ones128": ones128,
         "recip": recip}
        for b in range(B)
    ]
    res = run_bass_kernel_spmd(nc, in_maps, core_ids=list(range(B)), trace=trace)
    # un-reverse each 128-row block (device wrote them partition-flipped)
    out = np.stack([
        np.asarray(res.results[b]["out"])
        .reshape(NT, P, C)[:, ::-1, :]
        .reshape(T, C)
        .astype(np.float32)
        for b in range(B)
    ])
    return out, res


def kernel(x):
    out, _ = _run(x, trace=False)
    return out
